# revision 7
# baseline (speedup 1.0000x reference)
"""Multi-head attention (B=2, L=2048, H=16, D=64) on 8 TRN2 NeuronCores.

Sharding: core = (batch b, head-group hg); 2 batches x 4 groups of 4 heads.
All matmul inputs are bf16 (inputs cast host-side); PSUM accumulation f32.
rel_fro error of the all-bf16 pipeline ~4.6e-3 (vs 2e-2 budget).

Per core, for its batch and its 4 heads (2 head-pairs m):
    Q^T/K^T = W^T x^T           (d on partitions; pair m: head 2m at rows
                                 0:64, head 2m+1 at rows 64:128)
    V       = x W_v             (j on partitions, + ones column for denom)
    S^T     = K^T.T Q^T         (j on partitions, i free)
    es      = exp(S^T/8)        (bf16, un-normalized softmax numerator)
    O'^T    = [V|1].T es        (row 64 = softmax denominator)
    O^T     = O'[0:64] * (1/O'[64])
    out^T  += Wo_rows^T O^T     (partial over head-group rows of Wo)
Host sums the 4 partials per batch, transposes, adds bo.

Schedule: QK proj for m=0 -> V -> QK m=1 -> 4 attention units (ih, m) with
deferred normalize; wo chunks for ih=0 interleaved into units (1,*) j-loops;
tail: last normalizes + wo ih=1. Attention cadence is ACT(exp)-bound
(~2.3us per j step = 2 x [128,1024] exp tiles).
"""

import sys

try:
    import concourse.bass as bass  # noqa: F401
except ImportError:  # pragma: no cover - path fallback
    sys.path.insert(0, "/opt/trn_rl_repo")

import numpy as np
import ml_dtypes
import concourse.bass as bass
import concourse.mybir as mybir
import concourse.tile as tile
from concourse import bacc
from concourse.bass_utils import run_bass_kernel_spmd

F32 = mybir.dt.float32
BF16 = mybir.dt.bfloat16
AF = mybir.ActivationFunctionType
NPBF16 = ml_dtypes.bfloat16

B = 2
L = 2048          # sequence length
C = 1024          # model dim
H_LOC = 4         # heads per core
D = 64            # head dim
HD = H_LOC * D    # 256 = local head-group width
KT = C // 128     # 8 k-tiles over the model dim
SCALE2 = float(D) ** -0.5  # 1/8, applied once inside exp

_cache = {}


def _build():
    nc = bacc.Bacc("TRN2", target_bir_lowering=False, debug=False, num_devices=8)

    xT = nc.declare_dram_parameter("xT", [C, L], BF16, isOutput=False)
    wq = nc.declare_dram_parameter("wq", [C, HD], BF16, isOutput=False)
    wk = nc.declare_dram_parameter("wk", [C, HD], BF16, isOutput=False)
    wv = nc.declare_dram_parameter("wv", [C, HD], BF16, isOutput=False)
    wo = nc.declare_dram_parameter("wo", [HD, C], BF16, isOutput=False)
    outT = nc.declare_dram_parameter("outT", [C, L], F32, isOutput=True)

    with tile.TileContext(nc) as tc:
        with tc.tile_pool(name="sb", bufs=1) as sb:

            sbx = tc.alloc_tile_pool(name="sbx", bufs=1)

            # ---- load inputs (wq/wk first so projections start early) ------
            wq_sb = sb.tile([128, KT, HD], BF16, tag="wq")
            wk_sb = sb.tile([128, KT, HD], BF16, tag="wk")
            wv_sb = sb.tile([128, KT, HD], BF16, tag="wv")
            nc.sync.dma_start(wq_sb[:, :, :], wq.rearrange("(k p) c -> p k c", p=128))
            xT_sb = sbx.tile([128, KT, L], BF16, tag="xT")
            nc.sync.dma_start(xT_sb[:, 0, :], xT[0:128, :])
            nc.sync.dma_start(wk_sb[:, :, :], wk.rearrange("(k p) c -> p k c", p=128))
            for k in range(1, KT):
                nc.sync.dma_start(xT_sb[:, k, :], xT[k * 128:(k + 1) * 128, :])

            nc.sync.dma_start(wv_sb[:, :, :], wv.rearrange("(k p) c -> p k c", p=128))
            wo_sb = sb.tile([128, 2, C], BF16, tag="wo")
            nc.sync.dma_start(wo_sb[:, :, :], wo.rearrange("(k p) c -> p k c", p=128))

            ones_f = sb.tile([128, 64], BF16, tag="ones_f")
            nc.vector.memset(ones_f[:].bitcast(mybir.dt.uint16), 0x3F80)

            # ---- projections (PSUM supertiles, k-outer for LDW locality) ---
            qT_sb = sb.tile([128, 2, L], BF16, tag="qT")
            kT_sb = sb.tile([128, 2, L], BF16, tag="kT")

            pp = tc.alloc_tile_pool(name="pp", bufs=2, space="PSUM")

            def emit_qk_proj(m):
                for w_sb, t_sb in ((wq_sb, qT_sb), (wk_sb, kT_sb)):
                    p = pp.tile([128, 2048], F32, tag="qk")
                    for k in range(KT):
                        for n in range(4):
                            nc.tensor.matmul(
                                p[:, n * 512:(n + 1) * 512],
                                w_sb[:, k, m * 128:(m + 1) * 128],
                                xT_sb[:, k, n * 512:(n + 1) * 512],
                                start=(k == 0), stop=(k == KT - 1),
                            )
                    nc.vector.tensor_copy(t_sb[:, m, :], p[:])

            emit_qk_proj(0)

            # V with ones column: v_sb[p, j_tile, h, 0:64]=V, [..., 64]=1
            v_sb = sb.tile([128, 16, H_LOC, D + 1], BF16, tag="v")
            nc.vector.tensor_copy(
                v_sb[:, :, :, D:D + 1],
                ones_f.rearrange("p (a b c) -> p a b c", a=16, b=4),
            )
            for it in range(16):
                p = pp.tile([128, 2048], F32, tag="qk")
                acc = p[:, 0:HD]
                for k in range(KT):
                    nc.tensor.matmul(
                        acc,
                        xT_sb[:, k, it * 128:(it + 1) * 128],
                        wv_sb[:, k, :],
                        start=(k == 0), stop=(k == KT - 1),
                    )
                nc.vector.tensor_copy(
                    v_sb[:, it, :, 0:D],
                    acc.rearrange("p (h d) -> p h d", h=H_LOC),
                )

            emit_qk_proj(1)

            # x^T no longer needed; release pools for the attention phase
            pp.release()
            sbx.release()

            ps = tc.alloc_tile_pool(name="ps", bufs=2, space="PSUM")
            po = tc.alloc_tile_pool(name="po", bufs=2, space="PSUM")

            es_pool = tc.alloc_tile_pool(name="es_pool", bufs=6)
            st_pool = tc.alloc_tile_pool(name="st_pool", bufs=2)
            ost_pool = tc.alloc_tile_pool(name="ost_pool", bufs=6)
            np_pool = tc.alloc_tile_pool(name="np_pool", bufs=3)
            d0_pool = tc.alloc_tile_pool(name="d0_pool", bufs=1)

            # ---- attention + interleaved output projection ------------------
            oT_sb = sb.tile([128, 2, L], BF16, tag="oT")

            pending = []   # deferred normalize: (m, i0, [o_cp x2], [d0 x2])

            def emit_normalize():
                m, i0, o_cps, d0s = pending.pop(0)
                for hl in range(2):
                    rep_sb = st_pool.tile([64, 1024], F32, tag="rep")
                    nc.gpsimd.partition_broadcast(rep_sb[:], d0s[hl][:])
                    with nc.allow_low_precision(reason="bf16 matmul input"):
                        if hl == 0:
                            nc.vector.tensor_mul(
                                oT_sb[0:64, m, i0:i0 + 1024],
                                o_cps[hl][0:64, :], rep_sb[:])
                        else:
                            stage = st_pool.tile([64, 1024], BF16, tag="stage")
                            nc.vector.tensor_mul(
                                stage[:], o_cps[hl][0:64, :], rep_sb[:])
                            nc.gpsimd.dma_start(
                                oT_sb[64:128, m, i0:i0 + 1024], stage[:])

            def emit_wo_chunk(ih, ct):
                # [128, 1024] output chunk; kk-outer reuses each Wo k-tile's
                # weights across both 512-wide matmuls
                i0 = ih * 1024
                acc = ps.tile([128, 1024], F32, tag="s", name="wo_ps")
                for kk in range(2):
                    for n in range(2):
                        nc.tensor.matmul(
                            acc[:, n * 512:(n + 1) * 512],
                            wo_sb[:, kk, ct * 128:(ct + 1) * 128],
                            oT_sb[:, kk, i0 + n * 512:i0 + (n + 1) * 512],
                            start=(kk == 0), stop=(kk == 1),
                        )
                ost = ost_pool.tile([128, 1024], F32, tag="ost", name="ost")
                nc.vector.tensor_copy(ost[:], acc[:])
                nc.sync.dma_start(
                    outT[ct * 128:(ct + 1) * 128, i0:i0 + 1024], ost[:])

            units = [(ih, m) for ih in range(2) for m in range(2)]

            for ui, (ih, m) in enumerate(units):
                i0 = ih * 1024
                o_h = []
                for hl in range(2):
                    of = po.tile([128, 1024], F32, tag="o", name=f"o_ps{hl}")
                    o_h.append(of[0:65, :])
                es = [None, None]
                for j in range(16):
                    if j == 8 and pending:
                        emit_normalize()  # prior unit; reciprocal done by now
                    s_list = []
                    for hl in range(2):
                        r0 = hl * 64
                        s_ps = ps.tile([128, 1024], F32, tag="s", name=f"s_ps{hl}")
                        for n in range(2):
                            nc.tensor.matmul(
                                s_ps[:, n * 512:(n + 1) * 512],
                                kT_sb[r0:r0 + 64, m, j * 128:(j + 1) * 128],
                                qT_sb[r0:r0 + 64, m,
                                      i0 + n * 512:i0 + (n + 1) * 512],
                                start=True, stop=True,
                            )
                        s_list.append(s_ps)
                    prev_es = es
                    es = []
                    for hl in range(2):
                        e_sb = es_pool.tile([128, 1024], BF16, tag="es",
                                            name=f"es{hl}")
                        nc.scalar.activation(e_sb[:], s_list[hl][:], AF.Exp,
                                             scale=SCALE2)
                        es.append(e_sb)
                    # AV for step j-1 (software-pipelined one step behind)
                    if j > 0:
                        for hl in range(2):
                            for n in range(2):
                                nc.tensor.matmul(
                                    o_h[hl][:, n * 512:(n + 1) * 512],
                                    v_sb[:, j - 1, 2 * m + hl, :],
                                    prev_es[hl][:, n * 512:(n + 1) * 512],
                                    start=(j == 1), stop=False,
                                )
                # epilogue AV for j=15
                for hl in range(2):
                    for n in range(2):
                        nc.tensor.matmul(
                            o_h[hl][:, n * 512:(n + 1) * 512],
                            v_sb[:, 15, 2 * m + hl, :],
                            es[hl][:, n * 512:(n + 1) * 512],
                            start=False, stop=True,
                        )
                # pull O' off PSUM, reciprocal on the denominator row,
                # defer the normalize into the next unit's j-loop
                o_cps, d0s = [], []
                for hl in range(2):
                    o_cp = np_pool.tile([65, 1024], F32, tag="o_cp",
                                        name=f"o_cp{hl}")
                    nc.vector.tensor_copy(o_cp[:], o_h[hl][:])
                    # reshape the denominator row across all 128 lanes so the
                    # reciprocal runs at full DVE width, then reshape back
                    dsq = d0_pool.tile([128, 8], F32, tag=f"dsq_{hl}")
                    nc.gpsimd.dma_start(dsq[:], o_cp[64:65, :])
                    nc.vector.reciprocal(dsq[:], dsq[:])
                    d0 = d0_pool.tile([1, 1024], F32, tag=f"d0_{hl}")
                    nc.gpsimd.dma_start(d0[:], dsq[:])
                    o_cps.append(o_cp)
                    d0s.append(d0)
                pending.append((m, i0, o_cps, d0s))

            for ct in range(8):
                emit_wo_chunk(0, ct)
            while pending:
                emit_normalize()
            for ct in range(8):
                emit_wo_chunk(1, ct)

            d0_pool.release()
            np_pool.release()
            ost_pool.release()
            st_pool.release()
            es_pool.release()
            po.release()
            ps.release()

    nc.compile()
    return nc


def _prep_in_maps(x, Wq, Wk, Wv, Wo):
    xTs = [np.ascontiguousarray(x[b].T).astype(NPBF16) for b in range(B)]
    in_maps = []
    for core in range(8):
        b, hg = divmod(core, 4)
        sl = slice(hg * HD, (hg + 1) * HD)
        in_maps.append({
            "xT": xTs[b],
            "wq": np.ascontiguousarray(Wq[:, sl]).astype(NPBF16),
            "wk": np.ascontiguousarray(Wk[:, sl]).astype(NPBF16),
            "wv": np.ascontiguousarray(Wv[:, sl]).astype(NPBF16),
            "wo": np.ascontiguousarray(Wo[sl, :]).astype(NPBF16),
        })
    return in_maps


def kernel(x, Wq, Wk, Wv, Wo, bo):
    x = np.asarray(x, dtype=np.float32)
    Wq = np.asarray(Wq, dtype=np.float32)
    Wk = np.asarray(Wk, dtype=np.float32)
    Wv = np.asarray(Wv, dtype=np.float32)
    Wo = np.asarray(Wo, dtype=np.float32)
    bo = np.asarray(bo, dtype=np.float32)

    if "nc" not in _cache:
        _cache["nc"] = _build()
    nc = _cache["nc"]

    in_maps = _prep_in_maps(x, Wq, Wk, Wv, Wo)
    globals()["_last_in_maps"] = in_maps

    res = run_bass_kernel_spmd(nc, in_maps, core_ids=list(range(8)))
    out = np.empty((B, L, C), dtype=np.float32)
    for b in range(B):
        acc = res.results[4 * b]["outT"]
        for hg in range(1, 4):
            acc = acc + res.results[4 * b + hg]["outT"]
        out[b] = acc.T + bo
    return out


# revision 8
# speedup vs baseline: 1.0575x; 1.0575x over previous
"""Multi-head attention (B=2, L=2048, H=16, D=64) on 8 TRN2 NeuronCores.

Sharding: core = (batch b, head-group hg); 2 batches x 4 groups of 4 heads.
All matmul inputs bf16 (cast host-side), PSUM f32. rel_fro ~4.6e-3.

Dataflow per core (batch b, 4 heads = 2 pairs m, heads hl in pair):
    Q^T/K^T = W^T x^T        (d on partitions; head 2m at rows 0:64,
                              head 2m+1 at rows 64:128)
    V       = x W_v          (j on partitions, + ones column for denom)
    S^T     = K^T.T Q^T      (j on partitions, i free)
    es      = exp(S^T/8)     (bf16)
    O'^T    = [V|1].T es     (row 64 = denominator)
    O^T     = O'[0:64] * (1/O'[64])
    out^T  += Wo_rows^T O^T
Host sums the 4 partials per batch, transposes, adds bo.

Schedule: attention runs as 8 per-head units (ih, m, hl), each with its own
[65,1024] O accumulator (po bufs=1, 2 banks) and a 2-deep S ping-pong
(ps bufs=2, 4 banks), leaving 2 PSUM banks (pw) for work interleaved into
the ACT-bound attention slack: the m=1 QK projection runs inside units 0-1,
Wo chunks for i-half 0 inside units 5-7. The ACT engine (exp, ~1.1us per
[128,1024] tile) is the attention bottleneck; the PE fills its slack.
"""

import sys

try:
    import concourse.bass as bass  # noqa: F401
except ImportError:  # pragma: no cover - path fallback
    sys.path.insert(0, "/opt/trn_rl_repo")

import numpy as np
import ml_dtypes
import concourse.bass as bass
import concourse.mybir as mybir
import concourse.tile as tile
from concourse import bacc
from concourse.bass_utils import run_bass_kernel_spmd

F32 = mybir.dt.float32
BF16 = mybir.dt.bfloat16
AF = mybir.ActivationFunctionType
NPBF16 = ml_dtypes.bfloat16

B = 2
L = 2048          # sequence length
C = 1024          # model dim
H_LOC = 4         # heads per core
D = 64            # head dim
HD = H_LOC * D    # 256 = local head-group width
KT = C // 128     # 8 k-tiles over the model dim
SCALE2 = float(D) ** -0.5  # 1/8, applied once inside exp

_cache = {}


def _build():
    nc = bacc.Bacc("TRN2", target_bir_lowering=False, debug=False, num_devices=8)

    # all inputs host-prearranged to the SBUF layout (contiguous DMA)
    xT = nc.declare_dram_parameter("xT", [128, KT * L], BF16, isOutput=False)
    wq = nc.declare_dram_parameter("wq", [128, KT * HD], BF16, isOutput=False)
    wk = nc.declare_dram_parameter("wk", [128, KT * HD], BF16, isOutput=False)
    wv = nc.declare_dram_parameter("wv", [128, KT * HD], BF16, isOutput=False)
    wo = nc.declare_dram_parameter("wo", [128, 2 * C], BF16, isOutput=False)
    outT = nc.declare_dram_parameter("outT", [C, L], F32, isOutput=True)

    with tile.TileContext(nc) as tc:
        with tc.tile_pool(name="sb", bufs=1) as sb:

            # ---- load inputs (wq/wk first so projections start early) ------
            wq_sb = sb.tile([128, KT, HD], BF16, tag="wq")
            wk_sb = sb.tile([128, KT, HD], BF16, tag="wk")
            wv_sb = sb.tile([128, KT, HD], BF16, tag="wv")
            xT_sb = sb.tile([128, KT, L], BF16, tag="xT")
            nc.sync.dma_start(wq_sb[:, :, :], wq.rearrange("p (k c) -> p k c", k=KT))
            nc.sync.dma_start(wk_sb[:, :, :], wk.rearrange("p (k c) -> p k c", k=KT))
            for k in range(KT):
                nc.sync.dma_start(xT_sb[:, k, :], xT[:, k * L:(k + 1) * L])
            nc.sync.dma_start(wv_sb[:, :, :], wv.rearrange("p (k c) -> p k c", k=KT))
            wo_sb = sb.tile([128, 2, C], BF16, tag="wo")
            nc.sync.dma_start(wo_sb[:, :, :], wo.rearrange("p (k c) -> p k c", k=2))

            ones_f = sb.tile([128, 64], BF16, tag="ones_f")
            nc.vector.memset(ones_f[:].bitcast(mybir.dt.uint16), 0x3F80)

            qT_sb = sb.tile([128, 2, L], BF16, tag="qT")
            kT_sb = sb.tile([128, 2, L], BF16, tag="kT")

            # ---- QK m=0 projection + V (pp supertiles, k-outer) ------------
            pp = tc.alloc_tile_pool(name="pp", bufs=2, space="PSUM")

            def emit_qk_proj_pp(m):
                for w_sb, t_sb in ((wq_sb, qT_sb), (wk_sb, kT_sb)):
                    p = pp.tile([128, 2048], F32, tag="qk")
                    for k in range(KT):
                        for n in range(4):
                            nc.tensor.matmul(
                                p[:, n * 512:(n + 1) * 512],
                                w_sb[:, k, m * 128:(m + 1) * 128],
                                xT_sb[:, k, n * 512:(n + 1) * 512],
                                start=(k == 0), stop=(k == KT - 1),
                            )
                    nc.vector.tensor_copy(t_sb[:, m, :], p[:])

            emit_qk_proj_pp(0)

            # V with ones column: v_sb[p, j_tile, h, 0:64]=V, [..., 64]=1
            v_sb = sb.tile([128, 16, H_LOC, D + 1], BF16, tag="v")
            nc.vector.tensor_copy(
                v_sb[:, :, :, D:D + 1],
                ones_f.rearrange("p (a b c) -> p a b c", a=16, b=4),
            )
            for it in range(16):
                p = pp.tile([128, 2048], F32, tag="qk")
                acc = p[:, 0:HD]
                for k in range(KT):
                    nc.tensor.matmul(
                        acc,
                        xT_sb[:, k, it * 128:(it + 1) * 128],
                        wv_sb[:, k, :],
                        start=(k == 0), stop=(k == KT - 1),
                    )
                nc.vector.tensor_copy(
                    v_sb[:, it, :, 0:D],
                    acc.rearrange("p (h d) -> p h d", h=H_LOC),
                )

            pp.release()

            # ---- attention pools: 4 + 2 + 2 = 8 PSUM banks -----------------
            ps = tc.alloc_tile_pool(name="ps", bufs=2, space="PSUM")
            po = tc.alloc_tile_pool(name="po", bufs=1, space="PSUM")
            pw = tc.alloc_tile_pool(name="pw", bufs=1, space="PSUM")

            es_pool = tc.alloc_tile_pool(name="es_pool", bufs=4)
            st_pool = tc.alloc_tile_pool(name="st_pool", bufs=2)
            ost_pool = tc.alloc_tile_pool(name="ost_pool", bufs=4)
            np_pool = tc.alloc_tile_pool(name="np_pool", bufs=2)
            d0_pool = tc.alloc_tile_pool(name="d0_pool", bufs=2)

            oT_sb = sb.tile([128, 2, L], BF16, tag="oT")

            pending = []  # deferred normalize: (m, i0, hl, o_cp, d0)

            def emit_normalize():
                m, i0, hl, o_cp, d0 = pending.pop(0)
                rep_sb = st_pool.tile([64, 1024], F32, tag="rep")
                nc.gpsimd.partition_broadcast(rep_sb[:], d0[:])
                with nc.allow_low_precision(reason="bf16 matmul input"):
                    if hl == 0:
                        nc.vector.tensor_mul(
                            oT_sb[0:64, m, i0:i0 + 1024],
                            o_cp[0:64, :], rep_sb[:])
                    else:
                        stage = st_pool.tile([64, 1024], BF16, tag="stage")
                        nc.vector.tensor_mul(
                            stage[:], o_cp[0:64, :], rep_sb[:])
                        nc.gpsimd.dma_start(
                            oT_sb[64:128, m, i0:i0 + 1024], stage[:])

            def emit_o_drain(m, i0, hl, o_h):
                # pull O' off PSUM, reciprocal on the denominator row,
                # defer the PE-side normalize into the next unit's j-loop
                o_cp = np_pool.tile([65, 1024], F32, tag="o_cp")
                nc.vector.tensor_copy(o_cp[:], o_h[:])
                dsq = d0_pool.tile([128, 8], F32, tag="dsq")
                nc.gpsimd.dma_start(dsq[:], o_cp[64:65, :])
                nc.vector.reciprocal(dsq[:], dsq[:])
                d0 = d0_pool.tile([1, 1024], F32, tag="d0")
                nc.gpsimd.dma_start(d0[:], dsq[:])
                pending.append((m, i0, hl, o_cp, d0))

            def emit_wo_chunk(ih, ct, pool_tag):
                # [128, 1024] output chunk; kk-outer reuses Wo weights
                i0 = ih * 1024
                pool = pw if pool_tag == "w" else ps
                acc = pool.tile([128, 1024], F32, tag=pool_tag, name="wo_ps")
                for kk in range(2):
                    for n in range(2):
                        nc.tensor.matmul(
                            acc[:, n * 512:(n + 1) * 512],
                            wo_sb[:, kk, ct * 128:(ct + 1) * 128],
                            oT_sb[:, kk, i0 + n * 512:i0 + (n + 1) * 512],
                            start=(kk == 0), stop=(kk == 1),
                        )
                ost = ost_pool.tile([128, 1024], F32, tag="ost", name="ost")
                nc.vector.tensor_copy(ost[:], acc[:])
                nc.sync.dma_start(
                    outT[ct * 128:(ct + 1) * 128, i0:i0 + 1024], ost[:])

            # m=1 QK projection, interleaved into units 0-1 via pw tiles.
            # Each run: one [128,1024] n-half of q or k (16 matmuls).
            qk1_runs = [(w_sb, t_sb, nh) for w_sb, t_sb in
                        ((wq_sb, qT_sb), (wk_sb, kT_sb)) for nh in range(2)]

            def emit_qk1_run(run):
                w_sb, t_sb, nh = run
                acc = pw.tile([128, 1024], F32, tag="w", name="qk1")
                for k in range(KT):
                    for n2 in range(2):
                        nc.tensor.matmul(
                            acc[:, n2 * 512:(n2 + 1) * 512],
                            w_sb[:, k, 128:256],
                            xT_sb[:, k, nh * 1024 + n2 * 512:
                                  nh * 1024 + (n2 + 1) * 512],
                            start=(k == 0), stop=(k == KT - 1),
                        )
                nc.vector.tensor_copy(t_sb[:, 1, nh * 1024:(nh + 1) * 1024],
                                      acc[:])

            # units: i-half outer, then pair, then head-in-pair
            units = [(ih, m, hl) for ih in range(2) for m in range(2)
                     for hl in range(2)]
            wo0_queue = list(range(8))

            for ui, (ih, m, hl) in enumerate(units):
                i0 = ih * 1024
                r0 = hl * 64
                h = 2 * m + hl
                of = po.tile([128, 1024], F32, tag="o", name="o_ps")
                o_h = of[0:65, :]
                es_prev = None
                for j in range(16):
                    if j == 6 and pending:
                        emit_normalize()  # prior unit; reciprocal done by now
                    # interleaved background work in the ACT-bound slack
                    if ui < 2 and j == 0:
                        emit_qk1_run(qk1_runs[2 * ui])
                    if ui < 2 and j == 8:
                        emit_qk1_run(qk1_runs[2 * ui + 1])
                    if ui >= 5 and j in (2, 7, 12) and wo0_queue:
                        emit_wo_chunk(0, wo0_queue.pop(0), "w")
                    s_ps = ps.tile([128, 1024], F32, tag="s", name="s_ps")
                    for n in range(2):
                        nc.tensor.matmul(
                            s_ps[:, n * 512:(n + 1) * 512],
                            kT_sb[r0:r0 + 64, m, j * 128:(j + 1) * 128],
                            qT_sb[r0:r0 + 64, m,
                                  i0 + n * 512:i0 + (n + 1) * 512],
                            start=True, stop=True,
                        )
                    e_sb = es_pool.tile([128, 1024], BF16, tag="es", name="es")
                    nc.scalar.activation(e_sb[:], s_ps[:], AF.Exp, scale=SCALE2)
                    # AV for step j-1 (software-pipelined one step behind)
                    if j > 0:
                        for n in range(2):
                            nc.tensor.matmul(
                                o_h[:, n * 512:(n + 1) * 512],
                                v_sb[:, j - 1, h, :],
                                es_prev[:, n * 512:(n + 1) * 512],
                                start=(j == 1), stop=False,
                            )
                    es_prev = e_sb
                # epilogue AV for j=15
                for n in range(2):
                    nc.tensor.matmul(
                        o_h[:, n * 512:(n + 1) * 512],
                        v_sb[:, 15, h, :],
                        es_prev[:, n * 512:(n + 1) * 512],
                        start=False, stop=True,
                    )
                emit_o_drain(m, i0, hl, o_h)

            while wo0_queue:
                emit_wo_chunk(0, wo0_queue.pop(0), "w")
            while pending:
                emit_normalize()
            for ct in range(8):
                emit_wo_chunk(1, ct, "w" if ct % 2 == 0 else "s")

            d0_pool.release()
            np_pool.release()
            ost_pool.release()
            st_pool.release()
            es_pool.release()
            pw.release()
            po.release()
            ps.release()

    nc.compile()
    return nc


def _to_pk(a, kt):
    """[kt*128, c] -> [128, kt*c] host prearrangement for contiguous DMA."""
    c = a.shape[1]
    return np.ascontiguousarray(
        a.reshape(kt, 128, c).transpose(1, 0, 2).reshape(128, kt * c))


def _prep_in_maps(x, Wq, Wk, Wv, Wo):
    xTs = [_to_pk(np.ascontiguousarray(x[b].T), KT).astype(NPBF16)
           for b in range(B)]
    in_maps = []
    for core in range(8):
        b, hg = divmod(core, 4)
        sl = slice(hg * HD, (hg + 1) * HD)
        in_maps.append({
            "xT": xTs[b],
            "wq": _to_pk(np.ascontiguousarray(Wq[:, sl]), KT).astype(NPBF16),
            "wk": _to_pk(np.ascontiguousarray(Wk[:, sl]), KT).astype(NPBF16),
            "wv": _to_pk(np.ascontiguousarray(Wv[:, sl]), KT).astype(NPBF16),
            "wo": _to_pk(np.ascontiguousarray(Wo[sl, :]), 2).astype(NPBF16),
        })
    return in_maps


def kernel(x, Wq, Wk, Wv, Wo, bo):
    x = np.asarray(x, dtype=np.float32)
    Wq = np.asarray(Wq, dtype=np.float32)
    Wk = np.asarray(Wk, dtype=np.float32)
    Wv = np.asarray(Wv, dtype=np.float32)
    Wo = np.asarray(Wo, dtype=np.float32)
    bo = np.asarray(bo, dtype=np.float32)

    if "nc" not in _cache:
        _cache["nc"] = _build()
    nc = _cache["nc"]

    in_maps = _prep_in_maps(x, Wq, Wk, Wv, Wo)
    globals()["_last_in_maps"] = in_maps

    res = run_bass_kernel_spmd(nc, in_maps, core_ids=list(range(8)))
    out = np.empty((B, L, C), dtype=np.float32)
    for b in range(B):
        acc = res.results[4 * b]["outT"]
        for hg in range(1, 4):
            acc = acc + res.results[4 * b + hg]["outT"]
        out[b] = acc.T + bo
    return out
